# revision 1
# baseline (speedup 1.0000x reference)
"""Multi-head attention (B=2, N=2048, C=1024, H=16, D=64) on 8 TRN2 NeuronCores.

Sharding: 2 heads per core (tensor parallel over num_heads), both batch
elements processed on every core.  Each core computes q/k/v projections for
its 2 heads, full attention for those heads, and a partial output projection
(row-parallel over w_proj); the host sums the 8 partial outputs and adds the
bias.

Device-side dataflow per core:
  qkv:   qT/kT/vT [dpair=128, N] from xT tiles (c on partitions, f32r
         matmuls at full PE rate), accumulating over 8 c-tiles of 128.
         q/k are evacuated to bf16 with each head's 64 d-rows duplicated
         onto both partition halves, so score matmuls for two m-tiles can
         row-pack the PE array (rows 0:64 and 64:128 run concurrently).
  v:     vT -> bf16 -> PE transpose (128x128 tiles) -> vo tiles [m, d].
  attn:  per head, per m-tile pair: scores^T = kT_tile.T @ qT (K=64),
         exp via ACT (scale=1/8 folded in; no max-subtraction needed:
         logits are O(3) so fp32 exp is exact), writing bf16 E^T tiles;
         AV accumulation over m into PSUM, with a col-tiled ones matmul
         (cols 64:128 of the array) producing softmax denominators
         concurrently with the V matmul.
  norm:  reciprocal + cross-partition multiply into ocatT (f32r).
  proj:  y_partial[n, :] = ocatT.T @ w_projT, K=128 one-shot f32r matmuls.
"""

import sys

sys.path.insert(0, "/opt/trn_rl_repo")

import numpy as np

import concourse.bass as bass
import concourse.mybir as mybir
import concourse.tile as tile
from concourse import bacc
from concourse.bass_utils import run_bass_kernel_spmd
from concourse.masks import make_identity

F32 = mybir.dt.float32
F32R = mybir.dt.float32r
BF16 = mybir.dt.bfloat16
AF = mybir.ActivationFunctionType

B = 2
N = 2048
C = 1024
H = 16
D = 64
NCORES = 8
HPC = H // NCORES          # heads per core = 2
CT = C // 128              # c tiles = 8
NT = N // 128              # n/m tiles = 16
NCH = N // 512             # 512-wide n chunks = 4
SCALE = float(D) ** -0.5


def _build():
    nc = bacc.Bacc("TRN2")
    xT = nc.dram_tensor("xT", [B, C, N], F32R, kind="ExternalInput")
    wqkT = nc.dram_tensor("wqkT", [CT, 128, 256], F32R, kind="ExternalInput")
    wvT = nc.dram_tensor("wvT", [CT, 128, 128], F32R, kind="ExternalInput")
    wpT = nc.dram_tensor("wpT", [128, C], F32R, kind="ExternalInput")
    y = nc.dram_tensor("y", [B, N, C], F32, kind="ExternalOutput")

    with tile.TileContext(nc) as tc:
        with tc.tile_pool(name="consts", bufs=1) as consts, \
             tc.tile_pool(name="xt", bufs=8) as xt_pool, \
             tc.tile_pool(name="qk", bufs=8) as qk_pool, \
             tc.tile_pool(name="vt", bufs=2) as vt_pool, \
             tc.tile_pool(name="vo", bufs=2) as vo_pool, \
             tc.tile_pool(name="et", bufs=4) as et_pool, \
             tc.tile_pool(name="oc", bufs=2) as oc_pool, \
             tc.tile_pool(name="rec", bufs=2) as rec_pool, \
             tc.tile_pool(name="yo", bufs=4) as yo_pool, \
             tc.tile_pool(name="pbig", bufs=2, space="PSUM") as pbig, \
             tc.tile_pool(name="pav", bufs=4, space="PSUM") as pav:

            wqk_sb = consts.tile([128, CT, 256], F32R)
            wv_sb = consts.tile([128, CT, 128], F32R)
            wp_sb = consts.tile([128, C], F32R)
            ones_bf = consts.tile([128, 64], BF16)
            ident_bf = consts.tile([128, 128], BF16)
            nc.sync.dma_start(out=wqk_sb, in_=wqkT[:, :, :].rearrange("t p o -> p t o"))
            nc.sync.dma_start(out=wv_sb, in_=wvT[:, :, :].rearrange("t p o -> p t o"))
            nc.sync.dma_start(out=wp_sb, in_=wpT[:, :])
            nc.vector.memset(ones_bf, 1.0)
            make_identity(nc, ident_bf[:, :])

            for b in range(B):
                # ---- load xT tiles (c on partitions) ----
                xt = []
                for ct in range(CT):
                    t = xt_pool.tile([128, N], F32R, tag="xt", name=f"xt_{b}_{ct}")
                    nc.sync.dma_start(out=t, in_=xT[b, ct * 128:(ct + 1) * 128, :])
                    xt.append(t)

                # ---- q/k projections into duplicated-partition bf16 layout ----
                qd = [qk_pool.tile([128, N], BF16, tag="qk", name=f"qd_{b}_{h}")
                      for h in range(HPC)]
                kd = [qk_pool.tile([128, N], BF16, tag="qk", name=f"kd_{b}_{h}")
                      for h in range(HPC)]
                for ot, dsts in ((0, qd), (1, kd)):
                    for nch in range(NCH):
                        ps = pbig.tile([128, 512], F32, tag="pb",
                                       name=f"ps_{b}_{ot}_{nch}")
                        for ct in range(CT):
                            nc.tensor.matmul(
                                ps[:, :],
                                wqk_sb[:, ct, ot * 128:(ot + 1) * 128],
                                xt[ct][:, nch * 512:(nch + 1) * 512],
                                start=(ct == 0), stop=(ct == CT - 1),
                            )
                        sl = slice(nch * 512, (nch + 1) * 512)
                        for h in range(HPC):
                            src = ps[h * 64:(h + 1) * 64, :]
                            cp0 = nc.scalar.copy if b == 0 else nc.vector.tensor_copy
                            cp0(dsts[h][0:64, sl], src)
                            nc.vector.tensor_copy(dsts[h][64:128, sl], src)

                # ---- v projection (vT) + PE transpose to vo [m, d] ----
                vt_bf = vt_pool.tile([128, N], BF16, tag="vt", name=f"vt_{b}")
                for nch in range(NCH):
                    ps = pbig.tile([128, 512], F32, tag="pb", name=f"psv_{b}_{nch}")
                    for ct in range(CT):
                        nc.tensor.matmul(
                            ps[:, :],
                            wv_sb[:, ct, :],
                            xt[ct][:, nch * 512:(nch + 1) * 512],
                            start=(ct == 0), stop=(ct == CT - 1),
                        )
                    (nc.scalar.copy if b == 0 else nc.vector.tensor_copy)(
                        vt_bf[:, nch * 512:(nch + 1) * 512], ps[:, :])
                # vo layout per m-tile: [V_h0 (64) | ones (64) | V_h1 (64)] so each
                # head's AV stationary operand is a contiguous [V|ones] 128-col slab
                # (h0: cols 0:128 -> out = [O'; denom], h1: cols 64:192 -> [denom; O'])
                vo = vo_pool.tile([128, NT, 192], BF16, tag="vo", name=f"vo_{b}")
                nc.vector.memset(vo[:, :, 64:128], 1.0)
                for mt in range(NT):
                    tp = pbig.tile([128, 128], BF16, tag="pb", name=f"tp_{b}_{mt}")
                    nc.tensor.transpose(
                        tp[:, :], vt_bf[:, mt * 128:(mt + 1) * 128], ident_bf[:, :])
                    nc.vector.tensor_copy(vo[:, mt, 0:64], tp[:, 0:64])
                    nc.vector.tensor_copy(vo[:, mt, 128:192], tp[:, 64:128])

                oc_sb = oc_pool.tile([128, N], F32R, tag="oc", name=f"oc_{b}")

                # ---- attention per head (m-tiles processed in row-packed pairs) ----
                for hl in range(HPC):
                    hs = hl * 64
                    qdh, kdh = qd[hl], kd[hl]
                    avs = [pav.tile([128, 512], F32, tag="av",
                                    name=f"av_{b}_{hl}_{i}") for i in range(NCH)]
                    for j in range(NT // 2):
                        mA, mB = 2 * j, 2 * j + 1
                        # E^T for the pair, packed per n-quarter:
                        # et[:, q, 0:512] = E(mA, q-chunk), et[:, q, 512:1024] = E(mB, q-chunk)
                        et = et_pool.tile([128, NCH, 1024], BF16, tag="et",
                                          name=f"et_{b}_{hl}_{j}")
                        for q in range(NCH):
                            qof = q * 512
                            s = pbig.tile([128, 1024], F32, tag="pb",
                                          name=f"s_{b}_{hl}_{j}_{q}")
                            nc.tensor.matmul(
                                s[:, 0:512],
                                kdh[0:64, mA * 128:(mA + 1) * 128],
                                qdh[0:64, qof:qof + 512],
                                start=True, stop=True,
                            )
                            nc.tensor.matmul(
                                s[:, 512:1024],
                                kdh[64:128, mB * 128:(mB + 1) * 128],
                                qdh[64:128, qof:qof + 512],
                                start=True, stop=True,
                            )
                            nc.scalar.activation(out=et[:, q, :], in_=s[:, :],
                                                 func=AF.Exp, scale=SCALE)
                        for q in range(NCH):
                            for m_, eof in ((mA, 0), (mB, 512)):
                                nc.tensor.matmul(
                                    avs[q][:, :],
                                    vo[:, m_, hs:hs + 128],
                                    et[:, q, eof:eof + 512],
                                    start=(m_ == 0), stop=(m_ == NT - 1),
                                )
                    # h0: out partitions 0:64 = O', 64:128 = denom; h1 swapped
                    osl = slice(0, 64) if hl == 0 else slice(64, 128)
                    dsl = slice(64, 128) if hl == 0 else slice(0, 64)
                    for qq in range(NCH):
                        rec = rec_pool.tile([128, 512], F32, tag="rec",
                                            name=f"rec_{b}_{hl}_{qq}")
                        nc.vector.reciprocal(rec[dsl, :], avs[qq][dsl, :])
                        nc.vector.tensor_mul(
                            oc_sb[hs:hs + 64, qq * 512:(qq + 1) * 512],
                            avs[qq][osl, :],
                            rec[dsl, :],
                        )

                # ---- output projection (partial over this core's c-block) ----
                for nt in range(NT):
                    for och in range(2):
                        pp = pav.tile([128, 512], F32, tag="av",
                                      name=f"pp_{b}_{nt}_{och}")
                        nc.tensor.matmul(
                            pp[:, :],
                            oc_sb[:, nt * 128:(nt + 1) * 128],
                            wp_sb[:, och * 512:(och + 1) * 512],
                            start=True, stop=True,
                        )
                        ysb = yo_pool.tile([128, 512], F32, tag="yo",
                                           name=f"ysb_{b}_{nt}_{och}")
                        (nc.scalar.copy if (b == 1 and (nt + och) % 2 == 0)
                         else nc.vector.tensor_copy)(ysb[:, :], pp[:, :])
                        nc.sync.dma_start(
                            out=y[b, nt * 128:(nt + 1) * 128,
                                  och * 512:(och + 1) * 512],
                            in_=ysb[:, :],
                        )
    nc.finalize()
    return nc


_NC = None


def _get_nc():
    global _NC
    if _NC is None:
        _NC = _build()
    return _NC


def _make_in_maps(x, w_qkv):
    xT = np.ascontiguousarray(x.transpose(0, 2, 1)).astype(np.float32)
    in_maps = []
    for core in range(NCORES):
        h0 = core * HPC
        rows = np.concatenate(
            [np.arange(h * D, (h + 1) * D) for h in range(h0, h0 + HPC)]
        )
        wqk = np.concatenate([w_qkv[rows, :], w_qkv[C + rows, :]], axis=0)  # [256, 1024]
        wqkT = np.ascontiguousarray(wqk.T).reshape(CT, 128, 256)
        wvT = np.ascontiguousarray(w_qkv[2 * C + rows, :].T).reshape(CT, 128, 128)
        in_maps.append({"xT": xT, "wqkT": wqkT, "wvT": wvT})
    return in_maps


def kernel(x, w_qkv, w_proj, b_proj):
    x = np.asarray(x, dtype=np.float32)
    w_qkv = np.asarray(w_qkv, dtype=np.float32)
    w_proj = np.asarray(w_proj, dtype=np.float32)
    b_proj = np.asarray(b_proj, dtype=np.float32)

    in_maps = _make_in_maps(x, w_qkv)
    for core in range(NCORES):
        h0 = core * HPC
        cols = np.arange(h0 * D, (h0 + HPC) * D)
        in_maps[core]["wpT"] = np.ascontiguousarray(w_proj[:, cols].T)  # [128, 1024]

    nc = _get_nc()
    res = run_bass_kernel_spmd(nc, in_maps, core_ids=list(range(NCORES)))
    out = np.zeros((B, N, C), dtype=np.float32)
    for core in range(NCORES):
        out += res.results[core]["y"]
    out += b_proj
    return out



# revision 3
# speedup vs baseline: 1.4591x; 1.4591x over previous
"""Multi-head attention (B=2, N=2048, C=1024, H=16, D=64) on 8 TRN2 NeuronCores.

Sharding: 2 heads per core (tensor parallel over num_heads), both batch
elements on every core.  Each core computes q/k/v for its 2 heads, full
attention for those heads, and a partial output projection (row-parallel
over w_proj); the host sums the 8 partial outputs (f16) and adds the bias.

Device-side dataflow per core (per batch):
  qkv:   f16 matmuls over 8 c-tiles into [128,512] PSUM blocks
         (q block = 2 heads x 64d on partitions, same for k and v).
         q evacuated to fp8e4 duplicated into 2 DoubleRow slots;
         k evacuated as hi/lo compensated fp8e4 pair (k = khi + klo);
         v evacuated to fp8e4.
  vT:    PE transpose of v (fp8) into vo tiles [m, slot, V|ones|V] so each
         head's AV stationary operand is a [128, 2, 128] fp8 slab whose ones
         columns produce softmax denominators in the same matmul.
  attn:  per (head, n-chunk, m-pair): two fp8 DoubleRow score matmuls
         (slots = khi/klo versus q8 duplicated -> k fully compensated,
         q quantized) into a [128, 2, 512] PSUM tile; exp evacuation either
         on ACT (exact exp, fp8 out) or on DVE/Pool via a Schraudolph
         bit-trick (z = logit*8*log2e + 56.5-c as uint8, bitcast fp8e4m3);
         AV accumulation as one fp8 DoubleRow matmul per m-pair (slots =
         the two m-tiles) producing [O' | denom] per head.
  norm:  DVE reciprocal of denom + tensor-mul into oc (f16).
  proj:  y_partial[n, :] = oc.T @ w_proj in f16, evacuated f32->f16,
         DMA'd out per n-tile.

All matmul moving operands have free size 512 (DoubleRow: cost 256 rows);
fp8 DoubleRow halves PE time for scores and quarters it for AV.
"""

import sys

sys.path.insert(0, "/opt/trn_rl_repo")

import numpy as np
import ml_dtypes

import concourse.bass as bass
import concourse.mybir as mybir
import concourse.tile as tile
from concourse import bacc
from concourse.bass_utils import run_bass_kernel_spmd
from concourse.masks import make_identity

F32 = mybir.dt.float32
F16 = mybir.dt.float16
BF16 = mybir.dt.bfloat16
F8 = mybir.dt.float8e4
U8 = mybir.dt.uint8
AF = mybir.ActivationFunctionType
ALU = mybir.AluOpType
DR = mybir.MatmulPerfMode.DoubleRow

B = 2
N = 2048
C = 1024
H = 16
D = 64
NCORES = 8
HPC = H // NCORES          # heads per core = 2
CT = C // 128              # c tiles = 8
NT = N // 128              # m tiles = 16
NP = NT // 2               # m pairs = 8
NCH = N // 512             # 512-wide n chunks = 4
SCALE = float(D) ** -0.5

# Schraudolph exp constants: E ~ bitcast_u8(round(logit*SCALE*8*log2e + SB))
SA = SCALE * 8.0 / np.log(2.0)
SB = 56.0 - 0.4

# exp-tile engine plan: per (b, h, q, t) flattened index -> engine
# act = exact exp on scalar engine; dve/pool = Schraudolph bit trick
SCH_ENABLE = False

def _exp_engine(i):
    if not SCH_ENABLE:
        return "act"
    r = i % 16
    if r in (2, 7, 12):
        return "dve"
    if r in (4, 9, 14, 10):
        return "pool"
    return "act"


def _build():
    nc = bacc.Bacc("TRN2")
    xT = nc.dram_tensor("xT", [B, CT, 128, N], F16, kind="ExternalInput")
    wqkv = nc.dram_tensor("wqkv", [CT, 128, 384], F16, kind="ExternalInput")
    wpT = nc.dram_tensor("wpT", [128, C], F16, kind="ExternalInput")
    y = nc.dram_tensor("y", [B, N, C], F16, kind="ExternalOutput")

    with tile.TileContext(nc) as tc:
        with tc.tile_pool(name="consts", bufs=1) as consts, \
             tc.tile_pool(name="xt", bufs=16) as xt_pool, \
             tc.tile_pool(name="qk", bufs=4) as qk_pool, \
             tc.tile_pool(name="vt", bufs=2) as vt_pool, \
             tc.tile_pool(name="vo", bufs=2) as vo_pool, \
             tc.tile_pool(name="et", bufs=6) as et_pool, \
             tc.tile_pool(name="oc", bufs=2) as oc_pool, \
             tc.tile_pool(name="rec", bufs=4) as rec_pool, \
             tc.tile_pool(name="yo", bufs=6) as yo_pool, \
             tc.tile_pool(name="pq", bufs=2, space="PSUM") as pq, \
             tc.tile_pool(name="ps", bufs=2, space="PSUM") as ps_pool, \
             tc.tile_pool(name="pa", bufs=2, space="PSUM") as pa_pool:

            w_sb = consts.tile([128, CT, 384], F16)
            wp_sb = consts.tile([128, C], F16)
            ident8 = consts.tile([128, 128], F8)
            nc.sync.dma_start(out=w_sb, in_=wqkv[:, :, :].rearrange("t p o -> p t o"))
            nc.sync.dma_start(out=wp_sb, in_=wpT[:, :])
            make_identity(nc, ident8[:, :])

            ei = 0  # exp tile index for engine plan
            for b in range(B):
                # ---- load xT tiles (c on partitions) ----
                xt = []
                for ct in range(CT):
                    t = xt_pool.tile([128, N], F16, tag="xt", name=f"xt_{b}_{ct}")
                    nc.sync.dma_start(out=t, in_=xT[b, ct])
                    xt.append(t)

                # ---- q/k/v projections (f16), evacuate to fp8 layouts ----
                q8 = qk_pool.tile([128, 2, N], F8, tag="qk", name=f"q8_{b}")
                kk = qk_pool.tile([128, 2, N], F8, tag="qk", name=f"kk_{b}")
                vt8 = vt_pool.tile([128, N], F8, tag="vt", name=f"vt8_{b}")
                for blk, nch in ((i, j) for i in range(3) for j in range(NCH)):
                    psq = pq.tile([128, 512], F32, tag="pq", name=f"psq_{b}_{blk}_{nch}")
                    sl = slice(nch * 512, (nch + 1) * 512)
                    for ct in range(CT):
                        nc.tensor.matmul(
                            psq[:, :],
                            w_sb[:, ct, blk * 128:(blk + 1) * 128],
                            xt[ct][:, sl],
                            start=(ct == 0), stop=(ct == CT - 1),
                        )
                    if blk == 0:      # q: duplicate into both DoubleRow slots
                        nc.vector.tensor_copy(q8[:, 0, sl], psq[:, :])
                        nc.vector.tensor_copy(q8[:, 1, sl], psq[:, :])
                    elif blk == 1:    # k: hi/lo compensated split
                        nc.vector.tensor_copy(kk[:, 0, sl], psq[:, :])
                        nc.vector.tensor_tensor(
                            out=kk[:, 1, sl], in0=psq[:, :], in1=kk[:, 0, sl],
                            op=ALU.subtract)
                    else:             # v
                        nc.gpsimd.tensor_copy(vt8[:, sl], psq[:, :])

                # ---- v transpose into AV stationary layout ----
                # vo[m, pair, slot, 0:64]=V_h0, 64:128=ones, 128:192=V_h1
                vo = vo_pool.tile([128, NP, 2, 192], F8, tag="vo", name=f"vo_{b}")
                nc.gpsimd.memset(vo[:, :, :, 64:128], 1.0)
                for t in range(NP):
                    tp = pq.tile([128, 2, 128], F8, tag="pq", name=f"tp_{b}_{t}")
                    for s in range(2):
                        nc.tensor.transpose(
                            tp[:, s, :], vt8[:, (2 * t + s) * 128:(2 * t + s + 1) * 128],
                            ident8[:, :])
                    nc.gpsimd.tensor_copy(vo[:, t, :, 0:64], tp[:, :, 0:64])
                    nc.gpsimd.tensor_copy(vo[:, t, :, 128:192], tp[:, :, 64:128])

                oc_sb = oc_pool.tile([128, N], F16, tag="oc", name=f"oc_{b}")

                # ---- attention per (head, n-chunk): accumulate over m-pairs ----
                for hl in range(HPC):
                    hs = hl * 64
                    for q in range(NCH):
                        qof = q * 512
                        av = pa_pool.tile([128, 512], F32, tag="pa",
                                          name=f"av_{b}_{hl}_{q}")
                        for t in range(NP):
                            s = ps_pool.tile([128, 2, 512], F32, tag="ps",
                                             name=f"s_{b}_{hl}_{q}_{t}")
                            for i, m_ in enumerate((2 * t, 2 * t + 1)):
                                nc.tensor.matmul(
                                    s[:, i, :],
                                    kk[hs:hs + 64, :, m_ * 128:(m_ + 1) * 128],
                                    q8[hs:hs + 64, :, qof:qof + 512],
                                    start=True, stop=True, perf_mode=DR,
                                )
                            et = et_pool.tile([128, 2, 512], F8, tag="et",
                                              name=f"et_{b}_{hl}_{q}_{t}")
                            eng = _exp_engine(ei)
                            ei += 1
                            if eng == "act":
                                nc.scalar.activation(out=et[:, :, :], in_=s[:, :, :],
                                                     func=AF.Exp, scale=SCALE)
                            else:
                                v_eng = nc.vector if eng == "dve" else nc.gpsimd
                                v_eng.tensor_scalar(
                                    out=et[:, :, :].bitcast(U8), in0=s[:, :, :],
                                    scalar1=float(SA), scalar2=float(SB),
                                    op0=ALU.mult, op1=ALU.add)
                            nc.tensor.matmul(
                                av[:, :],
                                vo[:, t, :, hs:hs + 128],
                                et[:, :, :],
                                start=(t == 0), stop=(t == NP - 1), perf_mode=DR,
                            )
                        # h0: out parts 0:64 = O', 64:128 = denom; h1 swapped
                        osl = slice(0, 64) if hl == 0 else slice(64, 128)
                        dsl = slice(64, 128) if hl == 0 else slice(0, 64)
                        rec = rec_pool.tile([128, 512], F32, tag="rec",
                                            name=f"rec_{b}_{hl}_{q}")
                        nc.vector.reciprocal(rec[dsl, :], av[dsl, :])
                        nc.gpsimd.tensor_mul(
                            oc_sb[hs:hs + 64, qof:qof + 512],
                            av[osl, :],
                            rec[dsl, :],
                        )

                # ---- output projection (partial over this core's c-block) ----
                for nt in range(NT):
                    ysb = yo_pool.tile([128, 1024], F16, tag="yo",
                                       name=f"ysb_{b}_{nt}")
                    for och in range(2):
                        pp = pa_pool.tile([128, 512], F32, tag="pa",
                                          name=f"pp_{b}_{nt}_{och}")
                        nc.tensor.matmul(
                            pp[:, :],
                            oc_sb[:, nt * 128:(nt + 1) * 128],
                            wp_sb[:, och * 512:(och + 1) * 512],
                            start=True, stop=True,
                        )
                        cp = nc.vector.tensor_copy if (nt + och) % 2 == 0 \
                            else nc.gpsimd.tensor_copy
                        cp(ysb[:, och * 512:(och + 1) * 512], pp[:, :])
                    nc.sync.dma_start(
                        out=y[b, nt * 128:(nt + 1) * 128, :],
                        in_=ysb[:, :],
                    )
    nc.finalize()
    return nc


_NC = None


def _get_nc():
    global _NC
    if _NC is None:
        _NC = _build()
    return _NC


def _make_in_maps(x, w_qkv, w_proj):
    xT = np.ascontiguousarray(x.transpose(0, 2, 1)).astype(np.float16)
    xT = xT.reshape(B, CT, 128, N)
    in_maps = []
    for core in range(NCORES):
        h0 = core * HPC
        rows = np.concatenate(
            [np.arange(h * D, (h + 1) * D) for h in range(h0, h0 + HPC)]
        )
        w = np.concatenate(
            [w_qkv[rows, :], w_qkv[C + rows, :], w_qkv[2 * C + rows, :]], axis=0
        )  # [384, 1024]
        wqkvT = np.ascontiguousarray(w.T).astype(np.float16).reshape(CT, 128, 384)
        cols = np.arange(h0 * D, (h0 + HPC) * D)
        wpT = np.ascontiguousarray(w_proj[:, cols].T).astype(np.float16)
        in_maps.append({"xT": xT, "wqkv": wqkvT, "wpT": wpT})
    return in_maps


def kernel(x, w_qkv, w_proj, b_proj):
    x = np.asarray(x, dtype=np.float32)
    w_qkv = np.asarray(w_qkv, dtype=np.float32)
    w_proj = np.asarray(w_proj, dtype=np.float32)
    b_proj = np.asarray(b_proj, dtype=np.float32)

    in_maps = _make_in_maps(x, w_qkv, w_proj)
    nc = _get_nc()
    res = run_bass_kernel_spmd(nc, in_maps, core_ids=list(range(NCORES)))
    out = np.zeros((B, N, C), dtype=np.float32)
    for core in range(NCORES):
        out += res.results[core]["y"].astype(np.float32)
    out += b_proj
    return out
